# revision 25
# baseline (speedup 1.0000x reference)
"""MoE feed-forward (8 experts, top-2) Trainium2 kernel.

Strategy: data-parallel over the batch dim (B=8 -> one batch row of 4096
tokens per NeuronCore), expert weights replicated to all cores.  Each core
does on-device: router matmul -> top-2 gating -> position computation via a
triangular-matrix cumsum on the PE -> indirect-DMA scatter of token rows
into per-expert bins of a permuted DRAM buffer -> per-expert FFN GEMMs
(gelu) -> indirect-DMA gather + gate-weighted combine.

Per-expert bins have a static capacity of CAP rows (mean load is 1024 for
random routing; overflow tokens are clamped to a trash row, probability ~0
for any non-adversarial input).
"""

import os
import sys

import numpy as np

for _p in ("/opt/trn_rl_repo",):
    if _p not in sys.path:
        sys.path.insert(0, _p)

import concourse.bacc as bacc
import concourse.mybir as mybir
import concourse.tile as tile
from concourse import bass_utils, masks
from concourse.bass import IndirectOffsetOnAxis

F32 = mybir.dt.float32
F32R = mybir.dt.float32r
I32 = mybir.dt.int32
AF = mybir.ActivationFunctionType
ALU = mybir.AluOpType

# Problem shapes (hardcoded per contract)
B, T, D, FF, E = 8, 4096, 512, 2048, 8
N_CORES = 8
T_CORE = (B * T) // N_CORES          # 4096 tokens per core
NTT = T_CORE // 128                  # 32 token tiles
CAP = 1280                           # static per-expert row capacity
EBLK = 256                           # rows per expert GEMM block
NBLK = CAP // EBLK                   # 5
NPERM = E * CAP                      # 10240
TRASH = NPERM                        # trash row for (near-impossible) overflow
NPROWS = NPERM + 8

USE_F32R = True                      # full-rate fp32 matmul read mode


MM_DT = F32R if USE_F32R else F32


def build_nc(loop_iters=1, phases="REC"):
    """loop_iters>1 wraps the whole body in an on-device repeat loop, used
    only for timing (tunnel dispatch overhead cancels in the difference).
    phases: subset of "REC" (Routing/Experts/Combine) for timing attribution."""
    nc = bacc.Bacc("TRN2", target_bir_lowering=False, debug=False)

    x = nc.dram_tensor("x", [T_CORE, D], F32, kind="ExternalInput")
    wr = nc.dram_tensor("wr_t", [D, E], F32, kind="ExternalInput")
    w1 = nc.dram_tensor("w1_t", [E, D, FF], F32, kind="ExternalInput")
    w2 = nc.dram_tensor("w2_t", [E, FF, D], F32, kind="ExternalInput")
    basetile = nc.dram_tensor("basetile", [128, E], F32, kind="ExternalInput")
    out = nc.dram_tensor("out", [T_CORE, D], F32, kind="ExternalOutput")
    xperm = nc.dram_tensor("xperm", [NPROWS, D], F32)
    yperm = nc.dram_tensor("yperm", [NPROWS, D], F32)

    with tile.TileContext(nc) as tc:
        if loop_iters > 1:
            with tc.For_i(0, loop_iters, 1):
                _emit(nc, tc, x, wr, w1, w2, basetile, out, xperm, yperm,
                      phases)
        else:
            _emit(nc, tc, x, wr, w1, w2, basetile, out, xperm, yperm, phases)
    nc.compile()
    return nc


def _emit(nc, tc, x, wr, w1, w2, basetile, out, xperm, yperm, phases="REC"):
    with (
        tc.tile_pool(name="const", bufs=1) as cpool,
        tc.tile_pool(name="persist", bufs=1) as ppool,
    ):
        ident = cpool.tile([128, 128], F32)
        masks.make_identity(nc, ident[:])
        utri = cpool.tile([128, 128], F32)
        masks.make_upper_triangular(nc, utri[:], val=1.0, diag=True)
        ones_1x128 = cpool.tile([1, 128], F32)
        nc.vector.memset(ones_1x128[:], 1.0)
        ones_col = cpool.tile([128, 1], F32)
        nc.vector.memset(ones_col[:], 1.0)
        base_sb = cpool.tile([128, E], F32)
        nc.sync.dma_start(base_sb[:], basetile[:])
        wr_sb = cpool.tile([128, 4 * E], F32)  # 4 d-chunks of [128, 8]
        for k in range(4):
            nc.sync.dma_start(
                wr_sb[:, k * E:(k + 1) * E], wr[k * 128:(k + 1) * 128, :]
            )

        run = ppool.tile([1, E], F32)  # running per-expert counts
        nc.vector.memset(run[:], 0.0)
        # per-token-tile gather positions (int32) and gates, kept for combine
        posi = ppool.tile([128, 2 * NTT], I32)
        gates = ppool.tile([128, 2 * NTT], F32)

        # ---------------- Phase R: routing + dispatch ----------------
        if "R" not in phases:
            nc.vector.memset(posi[:], 0)
            nc.vector.memset(gates[:], 0.0)
        else:
         with (
            tc.tile_pool(name="r_sbuf", bufs=3) as rpool,
            tc.tile_pool(name="r_small", bufs=3) as spool,
            tc.tile_pool(name="r_psum", bufs=2, space="PSUM") as rps,
        ):
            for tt in range(NTT):
                # x rows duplicated side by side so ONE indirect scatter
                # covers both top-k destinations ([128, 2, 512] payload)
                xt = rpool.tile([128, 2 * D], F32, tag="xt")
                nc.sync.dma_start(xt[:, 0:D], x[tt * 128:(tt + 1) * 128, :])
                nc.scalar.dma_start(xt[:, D:2 * D],
                                    x[tt * 128:(tt + 1) * 128, :])

                # transpose x tile -> xT chunks [d128, t128] x4
                pst = rps.tile([128, D], F32, tag="pst")
                for k in range(4):
                    nc.tensor.transpose(
                        pst[:, k * 128:(k + 1) * 128],
                        xt[:, k * 128:(k + 1) * 128],
                        ident[:],
                    )

                xtt = rpool.tile([128, D], F32, tag="xtt")
                nc.scalar.copy(xtt[:], pst[:])

                # router logits [t128, e8]
                psl = rps.tile([128, E], F32, tag="psl")
                for k in range(4):
                    nc.tensor.matmul(
                        psl[:],
                        lhsT=xtt[:, k * 128:(k + 1) * 128],
                        rhs=wr_sb[:, k * E:(k + 1) * E],
                        start=(k == 0),
                        stop=(k == 3),
                    )
                lg = spool.tile([128, E], F32, tag="lg")
                nc.vector.tensor_copy(lg[:], psl[:])

                # top-2 + softmax gates
                m1 = spool.tile([128, 1], F32, tag="m1")
                nc.vector.reduce_max(m1[:], lg[:], axis=mybir.AxisListType.X)
                mask1 = spool.tile([128, E], F32, tag="mask1")
                nc.vector.tensor_scalar(
                    mask1[:], lg[:], m1[:], None, op0=ALU.is_equal
                )
                lg2 = spool.tile([128, E], F32, tag="lg2")
                # lg2 = lg - mask1 * 3e38  (knock out the max)
                nc.vector.scalar_tensor_tensor(
                    lg2[:], in0=mask1[:], scalar=-3e38, in1=lg[:],
                    op0=ALU.mult, op1=ALU.add,
                )
                m2 = spool.tile([128, 1], F32, tag="m2")
                nc.vector.reduce_max(m2[:], lg2[:], axis=mybir.AxisListType.X)
                mask2 = spool.tile([128, E], F32, tag="mask2")
                nc.vector.tensor_scalar(
                    mask2[:], lg2[:], m2[:], None, op0=ALU.is_equal
                )

                delta = spool.tile([128, 1], F32, tag="delta")
                nc.vector.tensor_sub(delta[:], m2[:], m1[:])
                expd = spool.tile([128, 1], F32, tag="expd")
                nc.scalar.activation(expd[:], delta[:], AF.Exp)
                den = spool.tile([128, 1], F32, tag="den")
                nc.vector.tensor_scalar_add(den[:], expd[:], 1.0)
                rcp = spool.tile([128, 1], F32, tag="rcp")
                nc.vector.reciprocal(rcp[:], den[:])
                g1 = gates[:, 2 * tt + 1:2 * tt + 2]
                nc.vector.tensor_tensor(g1, expd[:], rcp[:], op=ALU.mult)
                g0 = gates[:, 2 * tt:2 * tt + 1]
                nc.vector.tensor_tensor(g0, ones_col[:], g1, op=ALU.subtract)

                # cumsum of assignment mask -> per-expert relative positions
                m01 = spool.tile([128, E], F32, tag="m01")
                nc.vector.tensor_add(m01[:], mask1[:], mask2[:])
                psc = rps.tile([128, E], F32, tag="psc")
                nc.tensor.matmul(
                    psc[:], lhsT=utri[:], rhs=m01[:], start=True, stop=False
                )
                nc.tensor.matmul(
                    psc[:], lhsT=ones_1x128[:], rhs=run[:],
                    start=False, stop=True,
                )
                # update running counts += per-expert totals of this tile
                # (column sum via ones-vector matmul lands on partition 0;
                # reading psc[127] directly is an illegal partition base)
                pstot = rps.tile([1, E], F32, tag="pstot")
                nc.tensor.matmul(
                    pstot[:], lhsT=ones_col[:], rhs=m01[:],
                    start=True, stop=True,
                )
                nc.vector.tensor_add(run[:], run[:], pstot[:])

                relpos = spool.tile([128, E], F32, tag="relpos")
                nc.vector.tensor_sub(relpos[:], psc[:], m01[:])

                for kk, mk in ((0, mask1), (1, mask2)):
                    sel = spool.tile([128, E], F32, tag=f"sel{kk}")
                    nc.vector.tensor_tensor(
                        sel[:], relpos[:], mk[:], op=ALU.mult
                    )
                    rel = spool.tile([128, 1], F32, tag=f"rel{kk}")
                    nc.vector.reduce_sum(
                        rel[:], sel[:], axis=mybir.AxisListType.X
                    )
                    selb = spool.tile([128, E], F32, tag=f"selb{kk}")
                    nc.vector.tensor_tensor(
                        selb[:], base_sb[:], mk[:], op=ALU.mult
                    )
                    bsel = spool.tile([128, 1], F32, tag=f"bsel{kk}")
                    nc.vector.reduce_sum(
                        bsel[:], selb[:], axis=mybir.AxisListType.X
                    )
                    posf = spool.tile([128, 1], F32, tag=f"posf{kk}")
                    nc.vector.tensor_add(posf[:], rel[:], bsel[:])
                    # overflow (rel >= CAP) -> clamp to the trash row
                    ov = spool.tile([128, 1], F32, tag=f"ov{kk}")
                    nc.vector.tensor_scalar(
                        ov[:], rel[:], float(CAP), None, op0=ALU.is_ge
                    )
                    ovbig = spool.tile([128, 1], F32, tag=f"ovbig{kk}")
                    nc.vector.scalar_tensor_tensor(
                        ovbig[:], in0=ov[:], scalar=1e6, in1=posf[:],
                        op0=ALU.mult, op1=ALU.add,
                    )
                    posc = spool.tile([128, 1], F32, tag=f"posc{kk}")
                    nc.vector.tensor_scalar_min(posc[:], ovbig[:], float(TRASH))
                    nc.vector.tensor_copy(
                        posi[:, 2 * tt + kk:2 * tt + kk + 1], posc[:]
                    )

                # scatter the x rows to their two expert slots
                # (multi-column offset APs misbehave on HW ucode; keep
                # single-column offsets, one scatter per top-k slot)
                for kk in range(2):
                    nc.gpsimd.indirect_dma_start(
                        out=xperm[:, :],
                        out_offset=IndirectOffsetOnAxis(
                            ap=posi[:, 2 * tt + kk:2 * tt + kk + 1], axis=0
                        ),
                        in_=xt[:, kk * D:(kk + 1) * D],
                        in_offset=None,
                    )

        # ---------------- Phase E: per-expert FFN ----------------
        if "E" in phases:
         with (
            tc.tile_pool(name="e_w", bufs=2) as wpool,
            tc.tile_pool(name="e_sbuf", bufs=2) as epool,
            tc.tile_pool(name="e_psum", bufs=1, space="PSUM") as epsx,
            tc.tile_pool(name="e_psum_h", bufs=3, space="PSUM") as epsh,
            tc.tile_pool(name="e_psum_y", bufs=2, space="PSUM") as epsy,
        ):
            for e in range(E):
                # weights in MM_DT: for f32r the host pre-rounds, the DMA is a
                # bitcast byte move, and the PE reads at full (1 cyc/row) rate
                # weight DMAs issue from SP only: dma_start costs ~667ns of
                # the issuing engine's sequencer, and ACT's sequencer is on
                # the W1->gelu->W2 critical path
                w1sb = wpool.tile([128, 4 * FF], MM_DT, tag="w1")
                for k in range(4):
                    nc.sync.dma_start(
                        w1sb[:, k * FF:(k + 1) * FF],
                        w1[e, k * 128:(k + 1) * 128, :].bitcast(MM_DT),
                    )
                w2sb = wpool.tile([128, 16 * D], MM_DT, tag="w2")
                for k in range(16):
                    nc.sync.dma_start(
                        w2sb[:, k * D:(k + 1) * D],
                        w2[e, k * 128:(k + 1) * 128, :].bitcast(MM_DT),
                    )

                for blk in range(NBLK):
                    r0 = e * CAP + blk * EBLK
                    xa = epool.tile([128, D], F32, tag="xa")
                    nc.sync.dma_start(xa[:], xperm[r0:r0 + 128, :])
                    xb = epool.tile([128, D], F32, tag="xb")
                    nc.sync.dma_start(xb[:], xperm[r0 + 128:r0 + 256, :])

                    psxa = epsx.tile([128, D], F32, tag="psxa")
                    psxb = epsx.tile([128, D], F32, tag="psxb")
                    for k in range(4):
                        nc.tensor.transpose(
                            psxa[:, k * 128:(k + 1) * 128],
                            xa[:, k * 128:(k + 1) * 128],
                            ident[:],
                        )
                    for k in range(4):
                        nc.tensor.transpose(
                            psxb[:, k * 128:(k + 1) * 128],
                            xb[:, k * 128:(k + 1) * 128],
                            ident[:],
                        )
                    # xt2: [d128, 4 chunks x 256 tokens]; MM_DT output makes
                    # the ACT copy round to f32r as the ISA requires
                    xt2 = epool.tile([128, 4 * EBLK], MM_DT, tag="xt2")
                    xt2v = xt2[:].rearrange("p (c t) -> p c t", c=4)
                    psxav = psxa[:].rearrange("p (c t) -> p c t", c=4)
                    psxbv = psxb[:].rearrange("p (c t) -> p c t", c=4)
                    nc.scalar.copy(xt2v[:, :, 0:128], psxav)
                    nc.scalar.copy(xt2v[:, :, 128:256], psxbv)

                    # W1: hT [f 2048 (16 tiles), t 256] with exact gelu
                    ht = epool.tile([128, 16 * EBLK], MM_DT, tag="ht")
                    for ft in range(16):
                        psh = epsh.tile([128, EBLK], F32, tag="psh")
                        for k in range(4):
                            nc.tensor.matmul(
                                psh[:],
                                lhsT=w1sb[:, k * FF + ft * 128:
                                          k * FF + (ft + 1) * 128],
                                rhs=xt2[:, k * EBLK:(k + 1) * EBLK],
                                start=(k == 0),
                                stop=(k == 3),
                            )
                        nc.scalar.activation(
                            ht[:, ft * EBLK:(ft + 1) * EBLK], psh[:], AF.Gelu
                        )

                    # W2: y [t 128 x2, d 512]
                    for tsub in range(2):
                        psy = epsy.tile([128, D], F32, tag="psy")
                        for fc in range(16):
                            nc.tensor.matmul(
                                psy[:],
                                lhsT=ht[:, fc * EBLK + tsub * 128:
                                        fc * EBLK + tsub * 128 + 128],
                                rhs=w2sb[:, fc * D:(fc + 1) * D],
                                start=(fc == 0),
                                stop=(fc == 15),
                            )
                        ysb = epool.tile([128, D], F32, tag="ysb")
                        nc.vector.tensor_copy(ysb[:], psy[:])
                        nc.sync.dma_start(
                            yperm[r0 + tsub * 128:r0 + (tsub + 1) * 128, :],
                            ysb[:],
                        )

        # ---------------- Phase C: gather + gated combine ----------------
        if "C" in phases:
         with tc.tile_pool(name="c_sbuf", bufs=4) as c2pool:
            for tt in range(NTT):
                yab = c2pool.tile([128, 2 * D], F32, tag="yab")
                for kk in range(2):
                    nc.gpsimd.indirect_dma_start(
                        out=yab[:, kk * D:(kk + 1) * D],
                        out_offset=None,
                        in_=yperm[:, :],
                        in_offset=IndirectOffsetOnAxis(
                            ap=posi[:, 2 * tt + kk:2 * tt + kk + 1], axis=0
                        ),
                    )
                oa = c2pool.tile([128, D], F32, tag="oa")
                nc.vector.tensor_scalar_mul(
                    oa[:], yab[:, 0:D], gates[:, 2 * tt:2 * tt + 1]
                )
                ob = c2pool.tile([128, D], F32, tag="ob")
                nc.vector.tensor_scalar_mul(
                    ob[:], yab[:, D:2 * D], gates[:, 2 * tt + 1:2 * tt + 2]
                )
                nc.vector.tensor_add(oa[:], oa[:], ob[:])
                nc.scalar.dma_start(out[tt * 128:(tt + 1) * 128, :], oa[:])


_NC_CACHE = None
LAST_RESULTS = None


def _get_nc():
    global _NC_CACHE
    if _NC_CACHE is None:
        _NC_CACHE = build_nc()
    return _NC_CACHE


def kernel(x, Wr, W1, W2):
    global LAST_RESULTS
    x = np.ascontiguousarray(np.asarray(x, dtype=np.float32))
    wr_t = np.ascontiguousarray(np.asarray(Wr, np.float32).T)
    w1_t = np.ascontiguousarray(np.asarray(W1, np.float32).transpose(0, 2, 1))
    w2_t = np.ascontiguousarray(np.asarray(W2, np.float32).transpose(0, 2, 1))
    base = np.ascontiguousarray(
        np.broadcast_to(
            (np.arange(E, dtype=np.float32) * CAP)[None, :], (128, E)
        )
    )
    xs = x.reshape(N_CORES, T_CORE, D)
    nc = _get_nc()
    in_maps = [
        {"x": xs[c], "wr_t": wr_t, "w1_t": w1_t, "w2_t": w2_t,
         "basetile": base}
        for c in range(N_CORES)
    ]
    res = bass_utils.run_bass_kernel_spmd(
        nc, in_maps, core_ids=list(range(N_CORES))
    )
    LAST_RESULTS = res
    out = np.stack([res.results[c]["out"] for c in range(N_CORES)], axis=0)
    return out.reshape(B, T, D)


# revision 26
# speedup vs baseline: 1.2688x; 1.2688x over previous
"""MoE feed-forward (8 experts, top-2) Trainium2 kernel.

Strategy: data-parallel over the batch dim (B=8 -> one batch row of 4096
tokens per NeuronCore), expert weights replicated to all cores.  Each core
does on-device: router matmul -> top-2 gating -> position computation via a
triangular-matrix cumsum on the PE -> indirect-DMA scatter of token rows
into per-expert bins of a permuted DRAM buffer -> per-expert FFN GEMMs
(gelu) -> indirect-DMA gather + gate-weighted combine.

Per-expert bins have a static capacity of CAP rows (mean load is 1024 for
random routing; overflow tokens are clamped to a trash row, probability ~0
for any non-adversarial input).
"""

import os
import sys

import numpy as np

for _p in ("/opt/trn_rl_repo",):
    if _p not in sys.path:
        sys.path.insert(0, _p)

import concourse.bacc as bacc
import concourse.mybir as mybir
import concourse.tile as tile
from concourse import bass_utils, masks
from concourse.bass import IndirectOffsetOnAxis

F32 = mybir.dt.float32
F32R = mybir.dt.float32r
I32 = mybir.dt.int32
AF = mybir.ActivationFunctionType
ALU = mybir.AluOpType

# Problem shapes (hardcoded per contract)
B, T, D, FF, E = 8, 4096, 512, 2048, 8
N_CORES = 8
T_CORE = (B * T) // N_CORES          # 4096 tokens per core
NTT = T_CORE // 128                  # 32 token tiles
CAP = 1280                           # static per-expert row capacity
EBLK = 256                           # rows per expert GEMM block
NBLK = CAP // EBLK                   # 5
NPERM = E * CAP                      # 10240
TRASH = NPERM                        # trash row for (near-impossible) overflow
NPROWS = NPERM + 8

USE_F32R = True                      # full-rate fp32 matmul read mode


MM_DT = F32R if USE_F32R else F32


def build_nc(loop_iters=1, phases="REC"):
    """loop_iters>1 wraps the whole body in an on-device repeat loop, used
    only for timing (tunnel dispatch overhead cancels in the difference).
    phases: subset of "REC" (Routing/Experts/Combine) for timing attribution."""
    nc = bacc.Bacc("TRN2", target_bir_lowering=False, debug=False)

    x = nc.dram_tensor("x", [T_CORE, D], F32, kind="ExternalInput")
    wr = nc.dram_tensor("wr_t", [D, E], F32, kind="ExternalInput")
    w1 = nc.dram_tensor("w1_t", [E, D, FF], F32, kind="ExternalInput")
    w2 = nc.dram_tensor("w2_t", [E, FF, D], F32, kind="ExternalInput")
    basetile = nc.dram_tensor("basetile", [128, E], F32, kind="ExternalInput")
    out = nc.dram_tensor("out", [T_CORE, D], F32, kind="ExternalOutput")
    xperm = nc.dram_tensor("xperm", [NPROWS, D], F32)
    yperm = nc.dram_tensor("yperm", [NPROWS, D], F32)

    with tile.TileContext(nc) as tc:
        if loop_iters > 1:
            with tc.For_i(0, loop_iters, 1):
                _emit(nc, tc, x, wr, w1, w2, basetile, out, xperm, yperm,
                      phases)
        else:
            _emit(nc, tc, x, wr, w1, w2, basetile, out, xperm, yperm, phases)
    nc.compile()
    return nc


def _emit(nc, tc, x, wr, w1, w2, basetile, out, xperm, yperm, phases="REC"):
    with (
        tc.tile_pool(name="const", bufs=1) as cpool,
        tc.tile_pool(name="persist", bufs=1) as ppool,
    ):
        ident = cpool.tile([128, 128], F32)
        masks.make_identity(nc, ident[:])
        utri = cpool.tile([128, 128], F32)
        masks.make_upper_triangular(nc, utri[:], val=1.0, diag=True)
        ones_1x128 = cpool.tile([1, 128], F32)
        nc.vector.memset(ones_1x128[:], 1.0)
        ones_col = cpool.tile([128, 1], F32)
        nc.vector.memset(ones_col[:], 1.0)
        base_sb = cpool.tile([128, E], F32)
        nc.sync.dma_start(base_sb[:], basetile[:])
        wr_sb = cpool.tile([128, 4 * E], F32)  # 4 d-chunks of [128, 8]
        for k in range(4):
            nc.sync.dma_start(
                wr_sb[:, k * E:(k + 1) * E], wr[k * 128:(k + 1) * 128, :]
            )

        run = ppool.tile([1, E], F32)  # running per-expert counts
        nc.vector.memset(run[:], 0.0)
        # per-token-tile gather positions (int32) and gates, kept for combine
        posi = ppool.tile([128, 2 * NTT], I32)
        gates = ppool.tile([128, 2 * NTT], F32)

        # ---------------- Phase R: routing + dispatch ----------------
        if "R" not in phases:
            nc.vector.memset(posi[:], 0)
            nc.vector.memset(gates[:], 0.0)
        else:
         with (
            tc.tile_pool(name="r_sbuf", bufs=3) as rpool,
            tc.tile_pool(name="r_small", bufs=3) as spool,
            tc.tile_pool(name="r_psum", bufs=2, space="PSUM") as rps,
        ):
            for tt in range(NTT):
                # x rows duplicated side by side so ONE indirect scatter
                # covers both top-k destinations ([128, 2, 512] payload)
                xt = rpool.tile([128, 2 * D], F32, tag="xt")
                nc.sync.dma_start(xt[:, 0:D], x[tt * 128:(tt + 1) * 128, :])
                nc.scalar.dma_start(xt[:, D:2 * D],
                                    x[tt * 128:(tt + 1) * 128, :])

                # transpose x tile -> xT chunks [d128, t128] x4
                pst = rps.tile([128, D], F32, tag="pst")
                for k in range(4):
                    nc.tensor.transpose(
                        pst[:, k * 128:(k + 1) * 128],
                        xt[:, k * 128:(k + 1) * 128],
                        ident[:],
                    )

                xtt = rpool.tile([128, D], F32, tag="xtt")
                nc.scalar.copy(xtt[:], pst[:])

                # router logits [t128, e8]
                psl = rps.tile([128, E], F32, tag="psl")
                for k in range(4):
                    nc.tensor.matmul(
                        psl[:],
                        lhsT=xtt[:, k * 128:(k + 1) * 128],
                        rhs=wr_sb[:, k * E:(k + 1) * E],
                        start=(k == 0),
                        stop=(k == 3),
                    )
                lg = spool.tile([128, E], F32, tag="lg")
                nc.vector.tensor_copy(lg[:], psl[:])

                # top-2 + softmax gates
                m1 = spool.tile([128, 1], F32, tag="m1")
                nc.vector.reduce_max(m1[:], lg[:], axis=mybir.AxisListType.X)
                mask1 = spool.tile([128, E], F32, tag="mask1")
                nc.vector.tensor_scalar(
                    mask1[:], lg[:], m1[:], None, op0=ALU.is_equal
                )
                lg2 = spool.tile([128, E], F32, tag="lg2")
                # lg2 = lg - mask1 * 3e38  (knock out the max)
                nc.vector.scalar_tensor_tensor(
                    lg2[:], in0=mask1[:], scalar=-3e38, in1=lg[:],
                    op0=ALU.mult, op1=ALU.add,
                )
                m2 = spool.tile([128, 1], F32, tag="m2")
                nc.vector.reduce_max(m2[:], lg2[:], axis=mybir.AxisListType.X)
                mask2 = spool.tile([128, E], F32, tag="mask2")
                nc.vector.tensor_scalar(
                    mask2[:], lg2[:], m2[:], None, op0=ALU.is_equal
                )

                delta = spool.tile([128, 1], F32, tag="delta")
                nc.vector.tensor_sub(delta[:], m2[:], m1[:])
                expd = spool.tile([128, 1], F32, tag="expd")
                nc.scalar.activation(expd[:], delta[:], AF.Exp)
                den = spool.tile([128, 1], F32, tag="den")
                nc.vector.tensor_scalar_add(den[:], expd[:], 1.0)
                rcp = spool.tile([128, 1], F32, tag="rcp")
                nc.vector.reciprocal(rcp[:], den[:])
                g1 = gates[:, 2 * tt + 1:2 * tt + 2]
                nc.vector.tensor_tensor(g1, expd[:], rcp[:], op=ALU.mult)
                g0 = gates[:, 2 * tt:2 * tt + 1]
                nc.vector.tensor_tensor(g0, ones_col[:], g1, op=ALU.subtract)

                # cumsum of assignment mask -> per-expert relative positions
                m01 = spool.tile([128, E], F32, tag="m01")
                nc.vector.tensor_add(m01[:], mask1[:], mask2[:])
                psc = rps.tile([128, E], F32, tag="psc")
                nc.tensor.matmul(
                    psc[:], lhsT=utri[:], rhs=m01[:], start=True, stop=False
                )
                nc.tensor.matmul(
                    psc[:], lhsT=ones_1x128[:], rhs=run[:],
                    start=False, stop=True,
                )
                # update running counts += per-expert totals of this tile
                # (column sum via ones-vector matmul lands on partition 0;
                # reading psc[127] directly is an illegal partition base)
                pstot = rps.tile([1, E], F32, tag="pstot")
                nc.tensor.matmul(
                    pstot[:], lhsT=ones_col[:], rhs=m01[:],
                    start=True, stop=True,
                )
                nc.vector.tensor_add(run[:], run[:], pstot[:])

                relpos = spool.tile([128, E], F32, tag="relpos")
                nc.vector.tensor_sub(relpos[:], psc[:], m01[:])

                for kk, mk in ((0, mask1), (1, mask2)):
                    sel = spool.tile([128, E], F32, tag=f"sel{kk}")
                    nc.vector.tensor_tensor(
                        sel[:], relpos[:], mk[:], op=ALU.mult
                    )
                    rel = spool.tile([128, 1], F32, tag=f"rel{kk}")
                    nc.vector.reduce_sum(
                        rel[:], sel[:], axis=mybir.AxisListType.X
                    )
                    selb = spool.tile([128, E], F32, tag=f"selb{kk}")
                    nc.vector.tensor_tensor(
                        selb[:], base_sb[:], mk[:], op=ALU.mult
                    )
                    bsel = spool.tile([128, 1], F32, tag=f"bsel{kk}")
                    nc.vector.reduce_sum(
                        bsel[:], selb[:], axis=mybir.AxisListType.X
                    )
                    posf = spool.tile([128, 1], F32, tag=f"posf{kk}")
                    nc.vector.tensor_add(posf[:], rel[:], bsel[:])
                    # overflow (rel >= CAP) -> clamp to the trash row
                    ov = spool.tile([128, 1], F32, tag=f"ov{kk}")
                    nc.vector.tensor_scalar(
                        ov[:], rel[:], float(CAP), None, op0=ALU.is_ge
                    )
                    ovbig = spool.tile([128, 1], F32, tag=f"ovbig{kk}")
                    nc.vector.scalar_tensor_tensor(
                        ovbig[:], in0=ov[:], scalar=1e6, in1=posf[:],
                        op0=ALU.mult, op1=ALU.add,
                    )
                    posc = spool.tile([128, 1], F32, tag=f"posc{kk}")
                    nc.vector.tensor_scalar_min(posc[:], ovbig[:], float(TRASH))
                    nc.vector.tensor_copy(
                        posi[:, 2 * tt + kk:2 * tt + kk + 1], posc[:]
                    )

                # scatter the x rows to their two expert slots
                # (multi-column offset APs misbehave on HW ucode; keep
                # single-column offsets, one scatter per top-k slot)
                for kk in range(2):
                    nc.gpsimd.indirect_dma_start(
                        out=xperm[:, :],
                        out_offset=IndirectOffsetOnAxis(
                            ap=posi[:, 2 * tt + kk:2 * tt + kk + 1], axis=0
                        ),
                        in_=xt[:, kk * D:(kk + 1) * D],
                        in_offset=None,
                    )

        # ---------------- Phase E: per-expert FFN ----------------
        if "E" in phases:
         with (
            tc.tile_pool(name="e_w", bufs=2) as wpool,
            tc.tile_pool(name="e_sbuf", bufs=2) as epool,
            tc.tile_pool(name="e_psum", bufs=2, space="PSUM") as eps,
        ):
            for e in range(E):
                # weights in MM_DT: for f32r the host pre-rounds, the DMA is a
                # bitcast byte move, and the PE reads at full (1 cyc/row) rate
                # weight DMAs alternate between the two HWDGE queues
                # (SP/ACT) — each queue is a serial resource on HW, and the
                # ~64 MiB of weights dominates queue occupancy
                w1sb = wpool.tile([128, 4 * FF], MM_DT, tag="w1")
                for k in range(4):
                    eng = nc.sync if k % 2 == 0 else nc.scalar
                    eng.dma_start(
                        w1sb[:, k * FF:(k + 1) * FF],
                        w1[e, k * 128:(k + 1) * 128, :].bitcast(MM_DT),
                    )
                w2sb = wpool.tile([128, 16 * D], MM_DT, tag="w2")
                for k in range(16):
                    eng = nc.sync if k % 2 == 0 else nc.scalar
                    eng.dma_start(
                        w2sb[:, k * D:(k + 1) * D],
                        w2[e, k * 128:(k + 1) * 128, :].bitcast(MM_DT),
                    )

                for blk in range(NBLK):
                    r0 = e * CAP + blk * EBLK
                    xa = epool.tile([128, D], F32, tag="xa")
                    nc.sync.dma_start(xa[:], xperm[r0:r0 + 128, :])
                    xb = epool.tile([128, D], F32, tag="xb")
                    nc.scalar.dma_start(xb[:], xperm[r0 + 128:r0 + 256, :])

                    psxa = eps.tile([128, D], F32, tag="psxa")
                    psxb = eps.tile([128, D], F32, tag="psxb")
                    for k in range(4):
                        nc.tensor.transpose(
                            psxa[:, k * 128:(k + 1) * 128],
                            xa[:, k * 128:(k + 1) * 128],
                            ident[:],
                        )
                    for k in range(4):
                        nc.tensor.transpose(
                            psxb[:, k * 128:(k + 1) * 128],
                            xb[:, k * 128:(k + 1) * 128],
                            ident[:],
                        )
                    # xt2: [d128, 4 chunks x 256 tokens]; MM_DT output makes
                    # the ACT copy round to f32r as the ISA requires
                    xt2 = epool.tile([128, 4 * EBLK], MM_DT, tag="xt2")
                    xt2v = xt2[:].rearrange("p (c t) -> p c t", c=4)
                    psxav = psxa[:].rearrange("p (c t) -> p c t", c=4)
                    psxbv = psxb[:].rearrange("p (c t) -> p c t", c=4)
                    nc.scalar.copy(xt2v[:, :, 0:128], psxav)
                    nc.scalar.copy(xt2v[:, :, 128:256], psxbv)

                    # W1: hT [f 2048 (16 tiles), t 256] with exact gelu
                    ht = epool.tile([128, 16 * EBLK], MM_DT, tag="ht")
                    for ft in range(16):
                        psh = eps.tile([128, EBLK], F32, tag="psh")
                        for k in range(4):
                            nc.tensor.matmul(
                                psh[:],
                                lhsT=w1sb[:, k * FF + ft * 128:
                                          k * FF + (ft + 1) * 128],
                                rhs=xt2[:, k * EBLK:(k + 1) * EBLK],
                                start=(k == 0),
                                stop=(k == 3),
                            )
                        nc.scalar.activation(
                            ht[:, ft * EBLK:(ft + 1) * EBLK], psh[:], AF.Gelu
                        )

                    # W2: y [t 128 x2, d 512]
                    for tsub in range(2):
                        psy = eps.tile([128, D], F32, tag="psy")
                        for fc in range(16):
                            nc.tensor.matmul(
                                psy[:],
                                lhsT=ht[:, fc * EBLK + tsub * 128:
                                        fc * EBLK + tsub * 128 + 128],
                                rhs=w2sb[:, fc * D:(fc + 1) * D],
                                start=(fc == 0),
                                stop=(fc == 15),
                            )
                        ysb = epool.tile([128, D], F32, tag="ysb")
                        nc.vector.tensor_copy(ysb[:], psy[:])
                        eng = nc.sync if tsub == 0 else nc.scalar
                        eng.dma_start(
                            yperm[r0 + tsub * 128:r0 + (tsub + 1) * 128, :],
                            ysb[:],
                        )

        # ---------------- Phase C: gather + gated combine ----------------
        if "C" in phases:
         with tc.tile_pool(name="c_sbuf", bufs=4) as c2pool:
            for tt in range(NTT):
                yab = c2pool.tile([128, 2 * D], F32, tag="yab")
                for kk in range(2):
                    nc.gpsimd.indirect_dma_start(
                        out=yab[:, kk * D:(kk + 1) * D],
                        out_offset=None,
                        in_=yperm[:, :],
                        in_offset=IndirectOffsetOnAxis(
                            ap=posi[:, 2 * tt + kk:2 * tt + kk + 1], axis=0
                        ),
                    )
                oa = c2pool.tile([128, D], F32, tag="oa")
                nc.vector.tensor_scalar_mul(
                    oa[:], yab[:, 0:D], gates[:, 2 * tt:2 * tt + 1]
                )
                ob = c2pool.tile([128, D], F32, tag="ob")
                nc.vector.tensor_scalar_mul(
                    ob[:], yab[:, D:2 * D], gates[:, 2 * tt + 1:2 * tt + 2]
                )
                nc.vector.tensor_add(oa[:], oa[:], ob[:])
                nc.scalar.dma_start(out[tt * 128:(tt + 1) * 128, :], oa[:])


_NC_CACHE = None
LAST_RESULTS = None


def _get_nc():
    global _NC_CACHE
    if _NC_CACHE is None:
        _NC_CACHE = build_nc()
    return _NC_CACHE


def kernel(x, Wr, W1, W2):
    global LAST_RESULTS
    x = np.ascontiguousarray(np.asarray(x, dtype=np.float32))
    wr_t = np.ascontiguousarray(np.asarray(Wr, np.float32).T)
    w1_t = np.ascontiguousarray(np.asarray(W1, np.float32).transpose(0, 2, 1))
    w2_t = np.ascontiguousarray(np.asarray(W2, np.float32).transpose(0, 2, 1))
    base = np.ascontiguousarray(
        np.broadcast_to(
            (np.arange(E, dtype=np.float32) * CAP)[None, :], (128, E)
        )
    )
    xs = x.reshape(N_CORES, T_CORE, D)
    nc = _get_nc()
    in_maps = [
        {"x": xs[c], "wr_t": wr_t, "w1_t": w1_t, "w2_t": w2_t,
         "basetile": base}
        for c in range(N_CORES)
    ]
    res = bass_utils.run_bass_kernel_spmd(
        nc, in_maps, core_ids=list(range(N_CORES))
    )
    LAST_RESULTS = res
    out = np.stack([res.results[c]["out"] for c in range(N_CORES)], axis=0)
    return out.reshape(B, T, D)
